# revision 76
# baseline (speedup 1.0000x reference)
"""Trainium2 Bass kernel for nn_Abbott (delay-STDP weight update).

Math (d=8 delays, b=16 batch, N=1024 neurons):
    acc_p[b,e,o] = sum_d xbar_pre[d,b,e] * dmap[d,e,o]
    acc_d[b,e,o] = sum_d Xd[d,b,e]      * dmap[d,e,o]
    dW           = Xpost[b,o]*A_p[e,o]*acc_p - xbar_post[b,o]*A_d[e,o]*acc_d
    W_new        = clip(W + dW, 0, 1)
    xbar_pre'    = 0.95*xbar_pre + 0.05*Xd
    xbar_post'   = 0.9*xbar_post + 0.1*Xpost
    outputs: (W, W_new, xbar_pre', xbar_post')

Strategy: data-parallel over batch (2 batches/core on 8 cores).  The d-sum
runs on the TensorEngine as PSUM-accumulated fp8 DoubleRow matmuls with
DIAGONAL stationary matrices diag(xbar[d,b,e_tile]) — a diagonal matmul is a
per-partition (per-e) scale of dmap[d]; d-pairs are packed [K,2,*] for the
DoubleRow 2x rate and PSUM accumulates over the 4 pairs in fp32.  Diagonal
pair tiles are built on-chip on ScalarE (identity tile * per-partition
scalar); ScalarE also drains PSUM to SBUF (bf16).  VectorE runs the
elementwise epilogue, software-pipelined one e-tile behind the matmuls so PE
never stalls.  DMAs are consolidated and spread across the SP / ACT /
GpSimd queues (each dynamic-DGE queue serializes at ~3us per transfer, so
one queue cannot feed the kernel).  dmap travels as fp8 (a 0/1 mask — exact
in fp8); xbar diagonals are fp8 (the only lossy step, ~0.1% of output
scale on the ~1e-2-scale dW term); A/Xpost are bf16; W stays fp32.
"""

import numpy as np
import ml_dtypes

D, B, N = 8, 16, 1024
NCORES = 8
BPC = B // NCORES          # batches per core
P = 128                    # partitions
ET = N // P                # e-tiles per core
ALPHA_P = 0.95
ALPHA_D = 0.9
WMAX = 1.0

BF16 = ml_dtypes.bfloat16

_CACHE = {}

OUTC = BPC * N + D * BPC   # combined store: W_new columns + xbar_pre' columns


def _split_waits(nc, mybir):
    """Walrus in this toolchain encodes at most ONE sem-wait per instruction.
    Tile emits multi-wait instructions, so: (a) drop same-engine waits on
    serial compute-engine instructions (redundant — the engine executes one
    instruction at a time, in order; NOT valid for DMA issues, whose engine
    deps are real), (b) spill remaining extra waits into standalone
    EventSemaphore instructions just before the consumer, on the same engine
    queue (the same encoding Tile's own barriers use)."""
    prefix = {
        mybir.EngineType.DVE: "DVE_",
        mybir.EngineType.Activation: "ACT_",
        mybir.EngineType.PE: "PE_",
        mybir.EngineType.Pool: "POOL_",
    }
    fn = nc.m.functions[0]
    sync_info_cls = None
    for bb in fn.blocks:
        for ins in bb.instructions:
            if ins.sync_info is not None:
                sync_info_cls = type(ins.sync_info)
                break
        if sync_info_cls:
            break
    n_spill = 0
    for bb in fn.blocks:
        out = []
        for ins in bb.instructions:
            si = ins.sync_info
            ow = list(si.on_wait) if si and si.on_wait else []
            if len(ow) > 1 and "DMA" not in type(ins).__name__:
                pref = prefix.get(ins.engine)
                if pref is not None:
                    ow = [w for w in ow if not w.ant_name.startswith(pref)] or [ow[-1]]
            if len(ow) > 1:
                for j, w in enumerate(ow[:-1]):
                    ev = mybir.InstEventSemaphore(
                        name=f"{ins.name}_spill{j}", ins=[], outs=[]
                    )
                    ev.engine = ins.engine
                    ev.sync_info = sync_info_cls(on_wait=[w], on_update=[])
                    out.append(ev)
                    n_spill += 1
                ow = [ow[-1]]
            if si is not None:
                si.on_wait = ow
            out.append(ins)
        bb.instructions = out
    return n_spill


def _patch_dma_procs():
    """Pin each issuing engine's DMAs to its own DMA semaphore proc (SP->HW0,
    ACT->HW1, DVE->HW2, PE->HW3; Pool keeps SWDGE).  Each engine's dynamic
    queue is an in-order FIFO, so a per-queue semaphore counts completions
    correctly — and consumers then see at most one wait per queue."""
    import concourse.tile_sem_assignment as tsa
    import concourse.mybir as mybir
    if getattr(tsa, "_abbott_patched", False):
        return
    tsa._abbott_patched = True
    tsa.NUM_HWDGE_SEMS = 1  # unused by the patched path; keep deterministic

    orig = tsa.TileClockTick._assign_tick
    eng_to_hw = {
        mybir.EngineType.SP: 0,
        mybir.EngineType.Activation: 1,
        mybir.EngineType.DVE: 2,
        mybir.EngineType.PE: 3,
    }

    def _assign_tick(self, inst):
        eng = inst.engine
        if (
            isinstance(inst, tsa.DMAInst)
            and not isinstance(inst, tsa.bass_isa.UserSyncedRemoteDMADescs)
            and eng in eng_to_hw
        ):
            save = self.next_hw_dma_idx
            self.next_hw_dma_idx = eng_to_hw[eng]
            try:
                return orig(self, inst)
            finally:
                self.next_hw_dma_idx = save
        return orig(self, inst)

    tsa.TileClockTick._assign_tick = _assign_tick


def _build_nc():
    _patch_dma_procs()
    import concourse.bass as bass
    import concourse.mybir as mybir
    from concourse.tile import TileContext

    f32 = mybir.dt.float32
    bf16 = mybir.dt.bfloat16
    f8 = mybir.dt.float8e4
    mult = mybir.AluOpType.mult
    add = mybir.AluOpType.add
    sub = mybir.AluOpType.subtract
    amax = mybir.AluOpType.max
    amin = mybir.AluOpType.min

    nc = bass.Bass()

    # host-rearranged inputs (see kernel() marshalling)
    dm_in = nc.dram_tensor("dm_in", [ET, P, D * N], f8, kind="ExternalInput")
    id_in = nc.dram_tensor("id_in", [P, P], f8, kind="ExternalInput")
    apad_in = nc.dram_tensor("apad_in", [ET, P, 2 * N], bf16, kind="ExternalInput")
    w_in = nc.dram_tensor("w_in", [ET, P, BPC * N], f32, kind="ExternalInput")
    # consts: [xpb b0 | xbpb b0 | xpb b1 | xbpb b1]
    const_in = nc.dram_tensor("const_in", [P, 4 * N], bf16, kind="ExternalInput")
    xs_in = nc.dram_tensor("xs_in", [N, 2 * D * BPC], f32, kind="ExternalInput")
    xpp_in = nc.dram_tensor("xpp_in", [BPC, 2 * N], f32, kind="ExternalInput")

    out_all = nc.dram_tensor("out_all", [ET, P, OUTC], f32, kind="ExternalOutput")
    xbpn_out = nc.dram_tensor("xbpn_out", [BPC, N], f32, kind="ExternalOutput")

    NOC = 2          # 512-column chunks per accumulator (PSUM bank limit)
    OC = N // NOC

    with TileContext(nc) as tc:
        with (
            tc.tile_pool(name="const", bufs=1) as cpool,
            tc.tile_pool(name="dm", bufs=2) as dmpool,
            tc.tile_pool(name="dg", bufs=64) as dgpool,
            tc.tile_pool(name="wio", bufs=2) as wpool,
            tc.tile_pool(name="outp", bufs=3) as opool,
            tc.tile_pool(name="acc", bufs=2, space="PSUM") as psumpool,
            tc.tile_pool(name="accs", bufs=8) as accpool,
            tc.tile_pool(name="tmp", bufs=4) as tpool,
            tc.tile_pool(name="ab", bufs=2) as abpool,
            tc.tile_pool(name="sc", bufs=3) as scpool,
        ):
            id_tile = cpool.tile([P, P], f8, name="ident")
            nc.gpsimd.dma_start(out=id_tile[:], in_=id_in[:])
            id_t = id_tile[:]
            const_t = cpool.tile([P, 4 * N], bf16, name="const")
            nc.scalar.dma_start(out=const_t[:], in_=const_in[:])
            xpb_t = [const_t[:, 2 * b * N : (2 * b + 1) * N] for b in range(BPC)]
            xbpb_t = [const_t[:, (2 * b + 1) * N : (2 * b + 2) * N] for b in range(BPC)]

            def epilogue(st, split_store=False):
                p_et, w_t, ap_t, ad_t, xs_t, accs = st
                out_t = opool.tile([P, OUTC], f32, name="outt")
                for b in range(BPC):
                    pb = tpool.tile([P, N], bf16, name="pb")
                    nc.vector.tensor_tensor(out=pb[:], in0=ap_t, in1=xpb_t[b], op=mult)
                    db = tpool.tile([P, N], bf16, name="db")
                    nc.vector.tensor_tensor(out=db[:], in0=ad_t, in1=xbpb_t[b], op=mult)
                    pot = tpool.tile([P, N], bf16, name="pot")
                    nc.vector.tensor_tensor(out=pot[:], in0=accs[(b, 0)], in1=pb[:], op=mult)
                    dep = tpool.tile([P, N], bf16, name="dep")
                    nc.vector.tensor_tensor(out=dep[:], in0=accs[(b, 1)], in1=db[:], op=mult)
                    delta = tpool.tile([P, N], bf16, name="delta")
                    nc.vector.tensor_tensor(out=delta[:], in0=pot[:], in1=dep[:], op=sub)

                    ws = w_t[:, b * N : (b + 1) * N]
                    wns = out_t[:, b * N : (b + 1) * N]
                    nc.vector.tensor_tensor(out=wns, in0=ws, in1=delta[:], op=add)
                    nc.vector.tensor_scalar(
                        out=wns, in0=wns, scalar1=0.0, scalar2=WMAX,
                        op0=amax, op1=amin,
                    )
                    if split_store:
                        # tail latency: ship each batch's slice as soon as it
                        # clips instead of one store at the end
                        nc.gpsimd.dma_start(
                            out=out_all[p_et, :, b * N : (b + 1) * N], in_=wns
                        )

                # xbar_pre update (transposed layout), into the same store
                xc = BPC * N
                nc.vector.tensor_scalar(
                    out=out_t[:, xc : xc + D * BPC], in0=xs_t[:, : D * BPC],
                    scalar1=ALPHA_P, scalar2=None, op0=mult,
                )
                xn2_t = scpool.tile([P, D * BPC], f32, name="xspn2")
                nc.vector.tensor_scalar(
                    out=xn2_t[:], in0=xs_t[:, D * BPC :], scalar1=1.0 - ALPHA_P,
                    scalar2=None, op0=mult,
                )
                nc.vector.tensor_tensor(
                    out=out_t[:, xc : xc + D * BPC],
                    in0=out_t[:, xc : xc + D * BPC], in1=xn2_t[:], op=add,
                )
                if split_store:
                    xc = BPC * N
                    nc.gpsimd.dma_start(
                        out=out_all[p_et, :, xc:], in_=out_t[:, xc:]
                    )
                else:
                    nc.gpsimd.dma_start(out=out_all[p_et], in_=out_t[:])

            # xbar_post update (independent of the e-tile loop; runs in
            # the pipeline-fill idle time)
            xpp_t = cpool.tile([BPC, 2 * N], f32, name="xpp")
            nc.gpsimd.dma_start(out=xpp_t[:], in_=xpp_in[:])
            u_t = cpool.tile([BPC, N], f32, name="xbpn")
            nc.vector.tensor_scalar(
                out=u_t[:], in0=xpp_t[:, N:], scalar1=ALPHA_D, scalar2=None, op0=mult
            )
            u2_t = cpool.tile([BPC, N], f32, name="xbpn2")
            nc.vector.tensor_scalar(
                out=u2_t[:], in0=xpp_t[:, :N], scalar1=1.0 - ALPHA_D,
                scalar2=None, op0=mult,
            )
            nc.vector.tensor_tensor(out=u_t[:], in0=u_t[:], in1=u2_t[:], op=add)
            nc.gpsimd.dma_start(out=xbpn_out[:], in_=u_t[:])

            pending = None
            for et in range(ET):
                es = slice(et * P, (et + 1) * P)
                # dm + xs feed PE's critical path
                dm_t = dmpool.tile([P, D * N], f8, name="dm")
                if et == 0:
                    # split so the first matmuls start after half the load
                    nc.sync.dma_start(out=dm_t[:, : D * N // 2], in_=dm_in[et, :, : D * N // 2])
                    nc.sync.dma_start(out=dm_t[:, D * N // 2 :], in_=dm_in[et, :, D * N // 2 :])
                else:
                    nc.sync.dma_start(out=dm_t[:], in_=dm_in[et])
                xs_t = scpool.tile([P, 2 * D * BPC], f32, name="xs")
                nc.scalar.dma_start(out=xs_t[:], in_=xs_in[es, :])
                apad_t = abpool.tile([P, 2 * N], bf16, name="apad")
                nc.sync.dma_start(out=apad_t[:], in_=apad_in[et])
                w_t = wpool.tile([P, BPC * N], f32, name="wt")
                nc.scalar.dma_start(out=w_t[:], in_=w_in[et])

                # diag(xbar) pairs built on-chip (ACT): identity * per-
                # partition scalar, packed [K, 2, M] fp8 for DoubleRow
                dgs = {}
                for b in range(BPC):
                    for t in range(2):
                        # last-emitted group's diags go to DVE (it is busy
                        # with the previous epilogue anyway and ~1.5x faster
                        # per op); the other three groups to ACT, so each PE
                        # group waits on exactly one producer engine.  For the
                        # first e-tile both engines are idle — split evenly to
                        # shorten pipeline fill.
                        on_dve = (b == 0) if et == 0 else (b == BPC - 1 and t == 1)
                        for pd in range(D // 2):
                            dg_t = dgpool.tile([P, 2, P], f8, name="dgt")
                            for j in range(2):
                                d = 2 * pd + j
                                col = t * (D * BPC) + d * BPC + b
                                sc_ap = xs_t[:, col : col + 1]
                                if on_dve:
                                    nc.vector.tensor_scalar(
                                        out=dg_t[:, j], in0=id_t, scalar1=sc_ap,
                                        scalar2=None, op0=mult,
                                    )
                                else:
                                    nc.scalar.mul(dg_t[:, j], id_t, sc_ap)
                            dgs[(b, t, pd)] = dg_t

                # previous e-tile's epilogue AFTER this tile's diag builds, so
                # PE's next matmul burst is never blocked on DVE/ACT
                if pending is not None:
                    epilogue(pending, split_store=True)

                accs = {}
                for b in range(BPC):
                    # one 4-bank PSUM tile per batch: both terms' accumulators
                    # side by side -> a single wide ScalarE drain
                    acc = psumpool.tile([P, 2 * N], f32, name="acc")
                    for t in range(2):
                        for pd in range(D // 2):
                            for oc in range(NOC):
                                rhs = dm_t[
                                    :, 2 * pd * N : (2 * pd + 2) * N
                                ].rearrange("p (j o) -> p j o", j=2)
                                co = t * N + oc * OC
                                nc.tensor.matmul(
                                    acc[:, co : co + OC],
                                    dgs[(b, t, pd)][:],
                                    rhs[:, :, oc * OC : (oc + 1) * OC],
                                    start=(pd == 0),
                                    stop=(pd == D // 2 - 1),
                                    perf_mode=mybir.MatmulPerfMode.DoubleRow,
                                )
                    a_sb = accpool.tile([P, 2 * N], bf16, name="accs")
                    nc.scalar.copy(out=a_sb[:], in_=acc[:])
                    accs[(b, 0)] = a_sb[:, :N]
                    accs[(b, 1)] = a_sb[:, N:]

                pending = (et, w_t, apad_t[:, :N], apad_t[:, N:], xs_t, accs)

            epilogue(pending, split_store=True)

    _split_waits(nc, mybir)
    return nc


def _get_nc():
    if "nc" not in _CACHE:
        _CACHE["nc"] = _build_nc()
    return _CACHE["nc"]


def kernel(Xd, Xpost, W, xbar_pre, xbar_post, dmap, A_p, A_d):
    from concourse.bass_utils import run_bass_kernel_spmd

    nc = _get_nc()

    Xd = np.asarray(Xd, np.float32)
    Xpost = np.asarray(Xpost, np.float32)
    W = np.asarray(W, np.float32)
    xbar_pre = np.asarray(xbar_pre, np.float32)
    xbar_post = np.asarray(xbar_post, np.float32)
    dmap = np.asarray(dmap, np.float32)
    A_p = np.asarray(A_p, np.float32)
    A_d = np.asarray(A_d, np.float32)

    # shared (replicated) marshalling
    # dmap [d, e, o] -> [et, p, d*N+o]
    dm_shared = np.ascontiguousarray(
        dmap.reshape(D, ET, P, N).transpose(1, 2, 0, 3).reshape(ET, P, D * N)
    ).astype(ml_dtypes.float8_e4m3)
    # A_p | A_d -> [et, p, 2N]
    apad_host = np.ascontiguousarray(
        np.concatenate(
            [A_p.reshape(ET, P, N), A_d.reshape(ET, P, N)], axis=2
        ).astype(BF16)
    )

    in_maps = []
    for c in range(NCORES):
        bs = slice(c * BPC, (c + 1) * BPC)
        xsp = xbar_pre[:, bs, :]     # [d, bpc, e]
        xsd = Xd[:, bs, :]

        # W [b, e, o] -> [et, p, b*N+o]
        w_host = np.ascontiguousarray(
            W[bs].reshape(BPC, ET, P, N).transpose(1, 2, 0, 3).reshape(ET, P, BPC * N)
        )

        # consts: [xpb b0 | xbpb b0 | xpb b1 | xbpb b1 | identity]
        cols = []
        for b in range(BPC):
            cols.append(np.broadcast_to(Xpost[c * BPC + b][None, :], (P, N)))
            cols.append(np.broadcast_to(xbar_post[c * BPC + b][None, :], (P, N)))
        const_host = np.ascontiguousarray(np.concatenate(cols, axis=1).astype(BF16))

        # xbar traces transposed: [e, t*D*BPC + d*BPC + b]
        xs_host = np.ascontiguousarray(np.concatenate(
            [
                xsp.transpose(2, 0, 1).reshape(N, D * BPC),
                xsd.transpose(2, 0, 1).reshape(N, D * BPC),
            ],
            axis=1,
        ))

        in_maps.append({
            "dm_in": dm_shared,
            "id_in": np.ascontiguousarray(
                np.eye(P, dtype=np.float32).astype(ml_dtypes.float8_e4m3)
            ),
            "apad_in": apad_host,
            "w_in": w_host,
            "const_in": const_host,
            "xs_in": xs_host,
            "xpp_in": np.ascontiguousarray(
                np.concatenate([Xpost[bs], xbar_post[bs]], axis=1)
            ),
        })

    try:
        res = run_bass_kernel_spmd(
            nc, in_maps, core_ids=list(range(NCORES)), trace=False
        )
    except Exception:
        # the axon-proxied device occasionally reports a transient
        # UNAVAILABLE on first touch; poke it and retry once
        import jax
        import jax.numpy as jnp
        try:
            (jnp.ones((2, 2)) @ jnp.ones((2, 2))).block_until_ready()
        except Exception:
            pass
        res = run_bass_kernel_spmd(
            nc, in_maps, core_ids=list(range(NCORES)), trace=False
        )
    results = res.results

    # un-marshal: out_all [et, p, :BPC*N] -> W_new [b, e, o]; [:, BPC*N:] -> xbar_pre'
    W_new = np.concatenate(
        [
            r["out_all"][:, :, : BPC * N]
            .reshape(ET, P, BPC, N).transpose(2, 0, 1, 3).reshape(BPC, N, N)
            for r in results
        ],
        axis=0,
    )
    xbar_pre_new = np.concatenate(
        [
            r["out_all"][:, :, BPC * N :]
            .reshape(N, D, BPC).transpose(1, 2, 0)
            for r in results
        ],
        axis=1,
    )
    xbar_post_new = np.concatenate([r["xbpn_out"] for r in results], axis=0)

    out = np.array(W, dtype=np.float32, copy=True)
    return out, W_new, xbar_pre_new, xbar_post_new
